# revision 49
# baseline (speedup 1.0000x reference)
"""Conv1D (B=32, L=8192, C_in=64, K=3, F=128, VALID) + bias + ReLU on 8 trn2 cores.

Data-parallel over batch (4 batches per core). Quad-polyphase design:

  - Input loads 4 consecutive positions per partition (1KB descriptors) on
    the sync HWDGE ring; fp32->bf16 casts alternate scalar/DVE (gpsimd casts
    are 4x slower and stall the pipeline; swDGE cast-DMA costs 42ns/256B
    descriptor vs 29ns/512B hwDGE, so neither is used).
  - PE-transpose of each [128,(s,c)] sub-tile yields phases (0,1) or (2,3)
    stacked in partitions; strided-source full-128-partition DVE copies
    build xE[:,q]=[x[4q];x[4q+1]] and xO[:,q]=[x[4q+2];x[4q+3]].
  - 8 c=128 matmuls per 128-quad PSUM bank (2 per output phase; c=64
    matmuls hard-crash the device so single-tap matmuls pad the weight tile
    with zero rows):
      pos 4q   = xE[q]@[w0;w1] + xO[q]@[w2;0]
      pos 4q+1 = xE[q]@[0;w0]  + xO[q]@[w1;w2]
      pos 4q+2 = xO[q]@[w0;w1] + xE[q+1]@[w2;0]
      pos 4q+3 = xO[q]@[0;w0]  + xE[q+1]@[w1;w2]
  - PSUM bank [128,512] = 128 quads x (4 phases x F); ReLU (split
    scalar/DVE) writes bf16 staging; partition q holds positions 4q..4q+3
    so stores are 1KB descriptors; bf16 output halves write bytes (host
    upcasts to fp32; rel err ~4e-3 << 2e-2 budget).
  - Global software pipeline: fills run 2 chunks ahead of stores and
    interleave across batch boundaries (xE/xO halves alternate per batch).
"""

import os
import sys

import numpy as np

_TRN_REPO = "/opt/trn_rl_repo"
if _TRN_REPO not in sys.path and os.path.isdir(_TRN_REPO):
    sys.path.insert(0, _TRN_REPO)

import concourse.bass as bass
import concourse.tile as tile
from concourse import bacc, mybir
from concourse.bass_utils import run_bass_kernel_spmd
from concourse.masks import make_identity

B, L, C = 32, 8192, 64
K, F = 3, 128
L_OUT = L - K + 1  # 8190
N_CORES = 8
B_SHARD = B // N_CORES  # 4

MM_DT = mybir.dt.bfloat16
OUT_DT = mybir.dt.bfloat16

IN_CHUNK = 2048  # positions per input DMA chunk (512 quads)
N_CI = L // IN_CHUNK  # 4
QUADS = L // 4  # 2048 quads per batch (quad 2047 phases 2,3 are garbage)
XQ_PAD = 8
XQ_W = QUADS + XQ_PAD  # 2056
ST_QUADS = 512  # quads per output staging tile (4 banks of 128)
N_ST = QUADS // ST_QUADS  # 4 stores per batch


def _conv_kernel(tc: tile.TileContext, out_ap, x_ap, w_ap, b_ap, has_bias):
    nc = tc.nc
    fp32 = mybir.dt.float32

    with (
        tc.tile_pool(name="setup", bufs=1) as setup_pool,
        tc.tile_pool(name="xin", bufs=4) as xin_pool,
        tc.tile_pool(name="xbf", bufs=4) as xbf_pool,
        tc.tile_pool(name="osb", bufs=4) as osb_pool,
        tc.tile_pool(name="pt", bufs=3, space="PSUM") as pt_pool,
        tc.tile_pool(name="po", bufs=4, space="PSUM") as po_pool,
    ):
        # prefetch the first input chunks before any setup work so the DMA
        # engines start streaming during the weights/identity preamble
        prefetched = {}
        for pb, pci in ((0, 0), (0, 1)):
            xf = xin_pool.tile([128, 1024], fp32, name=f"xf_{pb}_{pci}", tag="xf")
            nc.sync.dma_start(
                out=xf.rearrange("p (t s c) -> p t s c", s=4, c=C),
                in_=x_ap[pb, pci * IN_CHUNK : (pci + 1) * IN_CHUNK, :].rearrange(
                    "(t p s) c -> p t s c", p=128, s=4
                ),
            )
            prefetched[(pb, pci)] = xf

        # --- one-time setup ---
        wstage = setup_pool.tile([C, K * F], fp32)
        for k in range(K):
            nc.scalar.dma_start(out=wstage[:, k * F : (k + 1) * F], in_=w_ap[k])

        # wAll cols: [0:F]=[w0;w1]  [F:2F]=[w1;w2]  [2F:3F]=[w2;0]  [3F:4F]=[0;w0]
        wAll = setup_pool.tile([128, 4 * F], MM_DT)
        nc.vector.memset(wAll[:, :], 0.0)
        nc.vector.tensor_copy(wAll[0:C, 0:F], wstage[:, 0:F])
        nc.vector.tensor_copy(wAll[C:128, 0:F], wstage[:, F : 2 * F])
        nc.vector.tensor_copy(wAll[0:C, F : 2 * F], wstage[:, F : 2 * F])
        nc.vector.tensor_copy(wAll[C:128, F : 2 * F], wstage[:, 2 * F : 3 * F])
        nc.vector.tensor_copy(wAll[0:C, 2 * F : 3 * F], wstage[:, 2 * F : 3 * F])
        nc.vector.tensor_copy(wAll[C:128, 3 * F : 4 * F], wstage[:, 0:F])

        if has_bias:
            bstage = setup_pool.tile([1, F], fp32)
            nc.scalar.dma_start(out=bstage[:, :], in_=b_ap[None, :])
            ones = setup_pool.tile([1, 128], MM_DT)
            nc.vector.memset(ones[:, :], 1.0)
            brow = setup_pool.tile([1, F], MM_DT)
            nc.vector.tensor_copy(brow[:, :], bstage[:, :])

        ident = setup_pool.tile([128, 128], MM_DT)
        make_identity(nc, ident)

        # xE/xO: manually double-buffered packed pair arrays.
        xE = setup_pool.tile([128, 2 * XQ_W], MM_DT)
        xO = setup_pool.tile([128, 2 * XQ_W], MM_DT)
        for h in range(2):
            nc.vector.memset(xE[:, h * XQ_W + QUADS : (h + 1) * XQ_W], 0.0)
            nc.vector.memset(xO[:, h * XQ_W + QUADS : (h + 1) * XQ_W], 0.0)

        relu_ctr = [0]
        store_ctr = [0]
        cast_ctr = [0]

        def fill_chunk(b, ci):
            xoff = (b % 2) * XQ_W
            c0 = ci * IN_CHUNK
            if (b, ci) in prefetched:
                xf = prefetched.pop((b, ci))
            else:
                xf = xin_pool.tile([128, 1024], fp32, name=f"xf_{b}_{ci}", tag="xf")
                nc.sync.dma_start(
                    out=xf.rearrange("p (t s c) -> p t s c", s=4, c=C),
                    in_=x_ap[b, c0 : c0 + IN_CHUNK, :].rearrange(
                        "(t p s) c -> p t s c", p=128, s=4
                    ),
                )
            xin = xbf_pool.tile([128, 1024], MM_DT, name=f"xin_{b}_{ci}", tag="xin")
            if cast_ctr[0] % 2 == 0:
                nc.scalar.copy(xin[:, :], xf[:, :])
            else:
                nc.vector.tensor_copy(xin[:, :], xf[:, :])
            cast_ctr[0] += 1

            pt8 = pt_pool.tile([128, 1024], MM_DT, name=f"pt8_{b}_{ci}", tag="pt8")
            for j in range(8):
                nc.tensor.transpose(
                    pt8[:, j * 128 : (j + 1) * 128],
                    xin[:, j * 128 : (j + 1) * 128],
                    ident,
                )
            # even sub-tiles hold [ph0;ph1] -> xE, odd hold [ph2;ph3] -> xO
            q = xoff + ci * (IN_CHUNK // 4)
            src = pt8.rearrange("p (t par h) -> p t par h", par=2, h=128)
            nc.vector.tensor_copy(
                xE[:, q : q + 512].rearrange("p (t h) -> p t h", h=128),
                src[:, :, 0, :],
            )
            nc.vector.tensor_copy(
                xO[:, q : q + 512].rearrange("p (t h) -> p t h", h=128),
                src[:, :, 1, :],
            )

        def store_chunk(b, s):
            xoff = (b % 2) * XQ_W
            o0 = s * 4 * ST_QUADS
            osb = osb_pool.tile(
                [128, 4 * ST_QUADS], OUT_DT, name=f"osb_{b}_{s}", tag="osb"
            )
            for bk in range(4):
                po = po_pool.tile([128, 512], fp32, name=f"po_{b}_{s}_{bk}", tag="po")
                q0 = xoff + s * ST_QUADS + bk * 128
                for ph in range(4):
                    c = ph * 128
                    lhs1 = xE[:, q0 : q0 + 128] if ph < 2 else xO[:, q0 : q0 + 128]
                    lhs2 = (
                        xO[:, q0 : q0 + 128]
                        if ph < 2
                        else xE[:, q0 + 1 : q0 + 129]
                    )
                    w1c = (0 if ph % 2 == 0 else 3) * F
                    w2c = (2 if ph % 2 == 0 else 1) * F
                    nc.tensor.matmul(
                        po[:, c : c + 128],
                        lhs1,
                        wAll[:, w1c : w1c + F],
                        start=True,
                        stop=False,
                    )
                    nc.tensor.matmul(
                        po[:, c : c + 128],
                        lhs2,
                        wAll[:, w2c : w2c + F],
                        start=False,
                        stop=not has_bias,
                    )
                    if has_bias:
                        nc.tensor.matmul(
                            po[:, c : c + 128],
                            ones[:, :],
                            brow[:, :],
                            start=False,
                            stop=True,
                        )
                ob = bk * 512
                # ReLU + fp32->bf16, split 5:3 scalar:DVE
                if relu_ctr[0] % 8 < 5:
                    nc.scalar.activation(
                        osb[:, ob : ob + 512],
                        po[:, :],
                        mybir.ActivationFunctionType.Relu,
                    )
                else:
                    nc.vector.tensor_scalar_max(osb[:, ob : ob + 512], po[:, :], 0.0)
                relu_ctr[0] += 1
                # stores: 1KB descriptors (4 consecutive positions); issue per
                # half-osb so the final drain is short, alternating rings. The
                # out tensor is padded to L=8192 so every store is a uniform
                # 256-descriptor DMA — small (<128-desc) tail DMAs land
                # entirely on a single DMA engine and serialize it. Positions
                # 8190/8191 hold relu(pad)=0 garbage; the host slices them off.
                if bk % 2 == 1:
                    g0 = bk - 1
                    eng = nc.scalar if store_ctr[0] % 2 == 0 else nc.sync
                    store_ctr[0] += 1
                    eng.dma_start(
                        out=out_ap[
                            b, o0 + g0 * 512 : o0 + (g0 + 2) * 512, :
                        ].rearrange("(g p s4) f -> p g s4 f", p=128, s4=4),
                        in_=osb[:, g0 * 512 : (g0 + 2) * 512].rearrange(
                            "p (g s4 f) -> p g s4 f", s4=4, f=F
                        ),
                    )

        # Software pipeline with a 2-fill lookahead: store (b,s) needs fill
        # (b,s+1) (the +1 window crosses one col into chunk s+1; s=3 needs
        # only fill 3 + the zero pad), which this order always satisfies.
        fills = [(b, ci) for b in range(B_SHARD) for ci in range(N_CI)]
        stores = [(b, s) for b in range(B_SHARD) for s in range(N_ST)]
        for f in fills[:2]:
            fill_chunk(*f)
        fi = 2
        for st in stores:
            if fi < len(fills):
                fill_chunk(*fills[fi])
                fi += 1
            store_chunk(*st)


def build_program(has_bias):
    nc = bacc.Bacc("TRN2", target_bir_lowering=False, debug=False)
    x = nc.dram_tensor("x", [B_SHARD, L, C], mybir.dt.float32, kind="ExternalInput")
    w = nc.dram_tensor("w", [K, C, F], mybir.dt.float32, kind="ExternalInput")
    bb = nc.dram_tensor("b", [F], mybir.dt.float32, kind="ExternalInput")
    out = nc.dram_tensor("out", [B_SHARD, L, F], OUT_DT, kind="ExternalOutput")
    with tile.TileContext(nc) as tc:
        _conv_kernel(tc, out.ap(), x.ap(), w.ap(), bb.ap(), has_bias)
    nc.compile()
    return nc


def kernel(x, w, b, _trace=False, _trace_kwargs=None):
    x = np.ascontiguousarray(np.asarray(x, dtype=np.float32))
    w = np.ascontiguousarray(np.asarray(w, dtype=np.float32))
    b = np.ascontiguousarray(np.asarray(b, dtype=np.float32))
    assert x.shape == (B, L, C) and w.shape == (K, C, F) and b.shape == (F,)

    nc = build_program(has_bias=bool(np.any(b)))
    in_maps = [
        {"x": x[i * B_SHARD : (i + 1) * B_SHARD], "w": w, "b": b}
        for i in range(N_CORES)
    ]
    res = run_bass_kernel_spmd(
        nc,
        in_maps,
        core_ids=list(range(N_CORES)),
        trace=_trace,
        **(_trace_kwargs or {}),
    )
    out = np.concatenate(
        [np.asarray(r["out"])[:, :L_OUT, :].astype(np.float32) for r in res.results],
        axis=0,
    )
    if _trace:
        return out, res
    return out


if __name__ == "__main__":
    rng = np.random.default_rng(0)
    x = rng.standard_normal((B, L, C), dtype=np.float32)
    w = rng.standard_normal((K, C, F), dtype=np.float32) * 0.08
    b = np.zeros((F,), dtype=np.float32)
    out = kernel(x, w, b)
    print("out", out.shape, out.dtype, float(np.abs(out).max()))


# revision 50
# speedup vs baseline: 1.0351x; 1.0351x over previous
"""Conv1D (B=32, L=8192, C_in=64, K=3, F=128, VALID) + bias + ReLU on 8 trn2 cores.

Data-parallel over batch (4 batches per core). Quad-polyphase design:

  - Input loads 4 consecutive positions per partition (1KB descriptors) on
    the sync HWDGE ring; fp32->bf16 casts alternate scalar/DVE (gpsimd casts
    are 4x slower and stall the pipeline; swDGE cast-DMA costs 42ns/256B
    descriptor vs 29ns/512B hwDGE, so neither is used).
  - PE-transpose of each [128,(s,c)] sub-tile yields phases (0,1) or (2,3)
    stacked in partitions; strided-source full-128-partition DVE copies
    build xE[:,q]=[x[4q];x[4q+1]] and xO[:,q]=[x[4q+2];x[4q+3]].
  - 8 c=128 matmuls per 128-quad PSUM bank (2 per output phase; c=64
    matmuls hard-crash the device so single-tap matmuls pad the weight tile
    with zero rows):
      pos 4q   = xE[q]@[w0;w1] + xO[q]@[w2;0]
      pos 4q+1 = xE[q]@[0;w0]  + xO[q]@[w1;w2]
      pos 4q+2 = xO[q]@[w0;w1] + xE[q+1]@[w2;0]
      pos 4q+3 = xO[q]@[0;w0]  + xE[q+1]@[w1;w2]
  - PSUM bank [128,512] = 128 quads x (4 phases x F); ReLU (split
    scalar/DVE) writes bf16 staging; partition q holds positions 4q..4q+3
    so stores are 1KB descriptors; bf16 output halves write bytes (host
    upcasts to fp32; rel err ~4e-3 << 2e-2 budget).
  - Global software pipeline: fills run 2 chunks ahead of stores and
    interleave across batch boundaries (xE/xO halves alternate per batch).
"""

import os
import sys

import numpy as np

_TRN_REPO = "/opt/trn_rl_repo"
if _TRN_REPO not in sys.path and os.path.isdir(_TRN_REPO):
    sys.path.insert(0, _TRN_REPO)

import concourse.bass as bass
import concourse.tile as tile
from concourse import bacc, mybir
from concourse.bass_utils import run_bass_kernel_spmd
from concourse.masks import make_identity

B, L, C = 32, 8192, 64
K, F = 3, 128
L_OUT = L - K + 1  # 8190
N_CORES = 8
B_SHARD = B // N_CORES  # 4

MM_DT = mybir.dt.bfloat16
OUT_DT = mybir.dt.bfloat16

IN_CHUNK = 2048  # positions per input DMA chunk (512 quads)
N_CI = L // IN_CHUNK  # 4
QUADS = L // 4  # 2048 quads per batch (quad 2047 phases 2,3 are garbage)
XQ_PAD = 8
XQ_W = QUADS + XQ_PAD  # 2056
ST_QUADS = 512  # quads per output staging tile (4 banks of 128)
N_ST = QUADS // ST_QUADS  # 4 stores per batch


def _conv_kernel(tc: tile.TileContext, out_ap, x_ap, w_ap, b_ap, has_bias):
    nc = tc.nc
    fp32 = mybir.dt.float32

    with (
        tc.tile_pool(name="setup", bufs=1) as setup_pool,
        tc.tile_pool(name="xin", bufs=4) as xin_pool,
        tc.tile_pool(name="xbf", bufs=4) as xbf_pool,
        tc.tile_pool(name="osb", bufs=4) as osb_pool,
        tc.tile_pool(name="pt", bufs=3, space="PSUM") as pt_pool,
        tc.tile_pool(name="po", bufs=4, space="PSUM") as po_pool,
    ):
        # prefetch the first input chunks before any setup work so the DMA
        # engines start streaming during the weights/identity preamble
        prefetched = {}
        for pb, pci in ((0, 0), (0, 1)):
            xf = xin_pool.tile([128, 1024], fp32, name=f"xf_{pb}_{pci}", tag="xf")
            nc.sync.dma_start(
                out=xf.rearrange("p (t s c) -> p t s c", s=4, c=C),
                in_=x_ap[pb, pci * IN_CHUNK : (pci + 1) * IN_CHUNK, :].rearrange(
                    "(t p s) c -> p t s c", p=128, s=4
                ),
            )
            prefetched[(pb, pci)] = xf

        # --- one-time setup ---
        wstage = setup_pool.tile([C, K * F], fp32)
        for k in range(K):
            nc.scalar.dma_start(out=wstage[:, k * F : (k + 1) * F], in_=w_ap[k])

        # wAll cols: [0:F]=[w0;w1]  [F:2F]=[w1;w2]  [2F:3F]=[w2;0]  [3F:4F]=[0;w0]
        wAll = setup_pool.tile([128, 4 * F], MM_DT)
        nc.vector.memset(wAll[:, :], 0.0)
        nc.vector.tensor_copy(wAll[0:C, 0:F], wstage[:, 0:F])
        nc.vector.tensor_copy(wAll[C:128, 0:F], wstage[:, F : 2 * F])
        nc.vector.tensor_copy(wAll[0:C, F : 2 * F], wstage[:, F : 2 * F])
        nc.vector.tensor_copy(wAll[C:128, F : 2 * F], wstage[:, 2 * F : 3 * F])
        nc.vector.tensor_copy(wAll[0:C, 2 * F : 3 * F], wstage[:, 2 * F : 3 * F])
        nc.vector.tensor_copy(wAll[C:128, 3 * F : 4 * F], wstage[:, 0:F])

        if has_bias:
            bstage = setup_pool.tile([1, F], fp32)
            nc.scalar.dma_start(out=bstage[:, :], in_=b_ap[None, :])
            ones = setup_pool.tile([1, 128], MM_DT)
            nc.vector.memset(ones[:, :], 1.0)
            brow = setup_pool.tile([1, F], MM_DT)
            nc.vector.tensor_copy(brow[:, :], bstage[:, :])

        ident = setup_pool.tile([128, 128], MM_DT)
        make_identity(nc, ident)

        # xE/xO: manually double-buffered packed pair arrays.
        xE = setup_pool.tile([128, 2 * XQ_W], MM_DT)
        xO = setup_pool.tile([128, 2 * XQ_W], MM_DT)
        for h in range(2):
            nc.vector.memset(xE[:, h * XQ_W + QUADS : (h + 1) * XQ_W], 0.0)
            nc.vector.memset(xO[:, h * XQ_W + QUADS : (h + 1) * XQ_W], 0.0)

        relu_ctr = [0]
        store_ctr = [0]
        cast_ctr = [0]

        def fill_chunk(b, ci):
            xoff = (b % 2) * XQ_W
            c0 = ci * IN_CHUNK
            if (b, ci) in prefetched:
                xf = prefetched.pop((b, ci))
            else:
                xf = xin_pool.tile([128, 1024], fp32, name=f"xf_{b}_{ci}", tag="xf")
                nc.sync.dma_start(
                    out=xf.rearrange("p (t s c) -> p t s c", s=4, c=C),
                    in_=x_ap[b, c0 : c0 + IN_CHUNK, :].rearrange(
                        "(t p s) c -> p t s c", p=128, s=4
                    ),
                )
            xin = xbf_pool.tile([128, 1024], MM_DT, name=f"xin_{b}_{ci}", tag="xin")
            if cast_ctr[0] % 2 == 0:
                nc.scalar.copy(xin[:, :], xf[:, :])
            else:
                nc.vector.tensor_copy(xin[:, :], xf[:, :])
            cast_ctr[0] += 1

            pt8 = pt_pool.tile([128, 1024], MM_DT, name=f"pt8_{b}_{ci}", tag="pt8")
            for j in range(8):
                nc.tensor.transpose(
                    pt8[:, j * 128 : (j + 1) * 128],
                    xin[:, j * 128 : (j + 1) * 128],
                    ident,
                )
            # even sub-tiles hold [ph0;ph1] -> xE, odd hold [ph2;ph3] -> xO
            q = xoff + ci * (IN_CHUNK // 4)
            src = pt8.rearrange("p (t par h) -> p t par h", par=2, h=128)
            nc.vector.tensor_copy(
                xE[:, q : q + 512].rearrange("p (t h) -> p t h", h=128),
                src[:, :, 0, :],
            )
            nc.vector.tensor_copy(
                xO[:, q : q + 512].rearrange("p (t h) -> p t h", h=128),
                src[:, :, 1, :],
            )

        def store_chunk(b, s):
            xoff = (b % 2) * XQ_W
            o0 = s * 4 * ST_QUADS
            osb = osb_pool.tile(
                [128, 4 * ST_QUADS], OUT_DT, name=f"osb_{b}_{s}", tag="osb"
            )
            for bk in range(4):
                po = po_pool.tile([128, 512], fp32, name=f"po_{b}_{s}_{bk}", tag="po")
                q0 = xoff + s * ST_QUADS + bk * 128
                for ph in range(4):
                    c = ph * 128
                    lhs1 = xE[:, q0 : q0 + 128] if ph < 2 else xO[:, q0 : q0 + 128]
                    lhs2 = (
                        xO[:, q0 : q0 + 128]
                        if ph < 2
                        else xE[:, q0 + 1 : q0 + 129]
                    )
                    w1c = (0 if ph % 2 == 0 else 3) * F
                    w2c = (2 if ph % 2 == 0 else 1) * F
                    nc.tensor.matmul(
                        po[:, c : c + 128],
                        lhs1,
                        wAll[:, w1c : w1c + F],
                        start=True,
                        stop=False,
                    )
                    nc.tensor.matmul(
                        po[:, c : c + 128],
                        lhs2,
                        wAll[:, w2c : w2c + F],
                        start=False,
                        stop=not has_bias,
                    )
                    if has_bias:
                        nc.tensor.matmul(
                            po[:, c : c + 128],
                            ones[:, :],
                            brow[:, :],
                            start=False,
                            stop=True,
                        )
                ob = bk * 512
                # ReLU + fp32->bf16, strict scalar/DVE alternation: the two
                # relus gating each half-store run on different engines in
                # parallel, and the 2-engine service rate (~381ns/bank) stays
                # ahead of the PE's 683ns/bank so po recycling never stalls
                if relu_ctr[0] % 2 == 0:
                    nc.scalar.activation(
                        osb[:, ob : ob + 512],
                        po[:, :],
                        mybir.ActivationFunctionType.Relu,
                    )
                else:
                    nc.vector.tensor_scalar_max(osb[:, ob : ob + 512], po[:, :], 0.0)
                relu_ctr[0] += 1
                # stores: 1KB descriptors (4 consecutive positions); issue per
                # half-osb so the final drain is short, alternating rings. The
                # out tensor is padded to L=8192 so every store is a uniform
                # 256-descriptor DMA — small (<128-desc) tail DMAs land
                # entirely on a single DMA engine and serialize it. Positions
                # 8190/8191 hold relu(pad)=0 garbage; the host slices them off.
                if bk % 2 == 1:
                    g0 = bk - 1
                    eng = nc.scalar if store_ctr[0] % 2 == 0 else nc.sync
                    store_ctr[0] += 1
                    eng.dma_start(
                        out=out_ap[
                            b, o0 + g0 * 512 : o0 + (g0 + 2) * 512, :
                        ].rearrange("(g p s4) f -> p g s4 f", p=128, s4=4),
                        in_=osb[:, g0 * 512 : (g0 + 2) * 512].rearrange(
                            "p (g s4 f) -> p g s4 f", s4=4, f=F
                        ),
                    )

        # Software pipeline with a 2-fill lookahead: store (b,s) needs fill
        # (b,s+1) (the +1 window crosses one col into chunk s+1; s=3 needs
        # only fill 3 + the zero pad), which this order always satisfies.
        fills = [(b, ci) for b in range(B_SHARD) for ci in range(N_CI)]
        stores = [(b, s) for b in range(B_SHARD) for s in range(N_ST)]
        for f in fills[:2]:
            fill_chunk(*f)
        fi = 2
        for st in stores:
            if fi < len(fills):
                fill_chunk(*fills[fi])
                fi += 1
            store_chunk(*st)


def build_program(has_bias):
    nc = bacc.Bacc("TRN2", target_bir_lowering=False, debug=False)
    x = nc.dram_tensor("x", [B_SHARD, L, C], mybir.dt.float32, kind="ExternalInput")
    w = nc.dram_tensor("w", [K, C, F], mybir.dt.float32, kind="ExternalInput")
    bb = nc.dram_tensor("b", [F], mybir.dt.float32, kind="ExternalInput")
    out = nc.dram_tensor("out", [B_SHARD, L, F], OUT_DT, kind="ExternalOutput")
    with tile.TileContext(nc) as tc:
        _conv_kernel(tc, out.ap(), x.ap(), w.ap(), bb.ap(), has_bias)
    nc.compile()
    return nc


def kernel(x, w, b, _trace=False, _trace_kwargs=None):
    x = np.ascontiguousarray(np.asarray(x, dtype=np.float32))
    w = np.ascontiguousarray(np.asarray(w, dtype=np.float32))
    b = np.ascontiguousarray(np.asarray(b, dtype=np.float32))
    assert x.shape == (B, L, C) and w.shape == (K, C, F) and b.shape == (F,)

    nc = build_program(has_bias=bool(np.any(b)))
    in_maps = [
        {"x": x[i * B_SHARD : (i + 1) * B_SHARD], "w": w, "b": b}
        for i in range(N_CORES)
    ]
    res = run_bass_kernel_spmd(
        nc,
        in_maps,
        core_ids=list(range(N_CORES)),
        trace=_trace,
        **(_trace_kwargs or {}),
    )
    out = np.concatenate(
        [np.asarray(r["out"])[:, :L_OUT, :].astype(np.float32) for r in res.results],
        axis=0,
    )
    if _trace:
        return out, res
    return out


if __name__ == "__main__":
    rng = np.random.default_rng(0)
    x = rng.standard_normal((B, L, C), dtype=np.float32)
    w = rng.standard_normal((K, C, F), dtype=np.float32) * 0.08
    b = np.zeros((F,), dtype=np.float32)
    out = kernel(x, w, b)
    print("out", out.shape, out.dtype, float(np.abs(out).max()))
